# revision 5
# baseline (speedup 1.0000x reference)
"""Causal multi-head attention block (qkv proj + attention + out proj) on 8 TRN2 cores.

Problem: x[4,2048,1024] @ Wqkv[1024,3072] -> 16-head causal attention -> @ Wout.

Sharding: batch(4) x head-group(2) -> 8 cores. Core c handles batch c//2 and
heads (c%2)*8..(c%2)*8+8. Each core computes its 8 heads' attention and a
partial out-projection [2048,1024]; host sums the two head-group partials per
batch and adds bout.

Design notes (measured on hw, ~300us vs 514us for the v1 kernel):
  - All matmul operands bf16: with other engines contending for SBUF ports,
    fp32r moving operands stream at ~0.6ns/col while bf16 holds the full
    ~0.42ns/col rate (and LDWEIGHTS halves to ~100ns, hiding under streams).
  - Causal slicing everywhere: diagonal kt blocks only compute/exp/accumulate
    the valid q-suffix (ctx accumulates kts ascending so partial-width
    accumulation is legal); masks shrink to one [128,128] triangle block.
  - exp ACTs batched over kt pairs ([128,2,512] PSUM spans, bf16 out); the
    narrow diagonal pair is exp'd per-slice. Scalar is the secondary
    bottleneck, so everything else stays off its queue.
  - Softmax denominator rides as V's 65th column through the ctx matmul;
    reciprocal on the [1,512] row (DVE), broadcast via GpSimd
    partition_broadcast (GpSimd latency is fine here because cn tiles are
    consumed one or more q-tiles later).
  - Projection + out-proj matmuls are deadline-scheduled into the attention
    inner loop via a paced background closure queue, so the PE has
    independent work wherever the mm_s -> exp -> mm_ctx chain would stall;
    y(0..2) all land in the qt3 phase, which is otherwise Scalar-bound.
  - PSUM: 2x[128,2,512] S pairs + 2x[128,512] proj + 2x[65,512] ctx = 8 banks.
  - Zero qkv bias assumed (asserted); bout added on host.
"""
import numpy as np

B, T, C = 4, 2048, 1024
H, HD = 16, 64
NCORES = 8


def _build_program():
    import concourse.bacc as bacc
    import concourse.tile as tile
    from concourse import mybir

    dtf = mybir.dt.float32
    dtr = mybir.dt.float32r
    dtb = mybir.dt.bfloat16
    EXP = mybir.ActivationFunctionType.Exp
    MULT = mybir.AluOpType.mult

    nc = bacc.Bacc('TRN2', target_bir_lowering=False, debug=False)
    xt_d = nc.dram_tensor("xt", [1024, 2048], dtb, kind="ExternalInput").ap()
    wqk_d = nc.dram_tensor("wqk", [1024, 1024], dtb, kind="ExternalInput").ap()
    wv_d = nc.dram_tensor("wv", [1024, 512], dtb, kind="ExternalInput").ap()
    wout_d = nc.dram_tensor("wout", [512, 1024], dtb, kind="ExternalInput").ap()
    mask_d = nc.dram_tensor("mask", [4, 128, 512], dtb, kind="ExternalInput").ap()
    y_d = nc.dram_tensor("y", [2048, 1024], dtf, kind="ExternalOutput").ap()

    with tile.TileContext(nc) as tc:
        with tc.tile_pool(name="ps_s", bufs=2, space="PSUM") as ps_s, \
             tc.tile_pool(name="ps", bufs=2, space="PSUM") as ps, \
             tc.tile_pool(name="ps_ctx", bufs=2, space="PSUM") as ps_ctx, \
             tc.tile_pool(name="const", bufs=1) as const, \
             tc.tile_pool(name="xt_p", bufs=16) as xt_p, \
             tc.tile_pool(name="qt_p", bufs=8) as qt_p, \
             tc.tile_pool(name="exp_p", bufs=8) as exp_p, \
             tc.tile_pool(name="cn_p", bufs=16) as cn_p, \
             tc.tile_pool(name="row_p", bufs=4) as row_p, \
             tc.tile_pool(name="rcp_p", bufs=2) as rcp_p, \
             tc.tile_pool(name="y_p", bufs=2) as y_p:

            # ---- constants / weights ----
            wqk_sb = const.tile([128, 8, 8, 128], dtb)   # [p, kc, oc, c]; oc 0-3 Q, 4-7 K
            wv_sb = const.tile([128, 8, 512], dtb)       # [p, kc, n]
            wout_sb = const.tile([128, 4, 2, 512], dtb)  # [p, hp, oc, c]
            masks = const.tile([128, 4, 512], dtb)
            ones_f32 = const.tile([1, 128], dtf)
            ones_t = const.tile([1, 128], dtr)
            kt_store = const.tile([128, 4, 4, 512], dtb)  # [p, j, tt, t]
            v_all = const.tile([128, 16, 8, 65], dtb)     # [p, kt, h, d|1]

            nc.vector.memset(v_all[:, :, :, 64:65], 1.0)
            nc.vector.memset(ones_f32[:], 1.0)
            nc.vector.tensor_copy(ones_t[:], ones_f32[:])

            qts = {}   # tt -> [4 qt tiles]
            xts = {}   # tt -> [8 xt tiles]
            cns = {}   # qt -> [4 cn tiles]

            def emit_xt_dma(tt):
                ts = []
                for kc in range(8):
                    t_ = xt_p.tile([128, 512], dtb, tag="xt")
                    nc.sync.dma_start(
                        t_[:], xt_d[kc * 128:(kc + 1) * 128,
                                    tt * 512:(tt + 1) * 512])
                    ts.append(t_)
                xts[tt] = ts

            # DMA order: (wqk[kc], xt0[kc]) interleaved so Q(0) can start
            # ~1us in; then masks (qt0 attention), wv (V(0)), xt1, wout (y(0)).
            xts[0] = []
            for kc in range(8):
                nc.sync.dma_start(wqk_sb[:, kc, :, :],
                                  wqk_d[kc * 128:(kc + 1) * 128, :]
                                  .rearrange("p (oc c) -> p oc c", c=128))
                t_ = xt_p.tile([128, 512], dtb, tag="xt", name="xt0")
                nc.scalar.dma_start(
                    t_[:], xt_d[kc * 128:(kc + 1) * 128, 0:512])
                xts[0].append(t_)
            nc.sync.dma_start(masks[:], mask_d.rearrange("n p f -> p n f"))
            nc.sync.dma_start(wv_sb[:],
                              wv_d.rearrange("(kc p) n -> p kc n", p=128))
            emit_xt_dma(1)
            nc.sync.dma_start(wout_sb[:],
                              wout_d.rearrange("(hp p) (oc c) -> p hp oc c",
                                               p=128, c=512))

            def proj_closures(tt):
                cl = []
                qts[tt] = [None] * 4
                state = {}

                def q_mm(j, kc):
                    if kc == 0:
                        state[('q', j)] = ps.tile([128, 512], dtf, tag="ps", name="psq")
                    psq = state[('q', j)]
                    nc.tensor.matmul(psq[:], wqk_sb[:, kc, j, :],
                                     xts[tt][kc][:], start=(kc == 0),
                                     stop=(kc == 7)).annotate('mm_q')
                    if kc == 7:
                        qt_t = qt_p.tile([128, 512], dtb, tag="qt")
                        nc.vector.tensor_copy(qt_t[:], psq[:])
                        qts[tt][j] = qt_t

                def k_mm(j, kc):
                    if kc == 0:
                        state[('k', j)] = ps.tile([128, 512], dtf, tag="ps", name="psk")
                    psk = state[('k', j)]
                    nc.tensor.matmul(psk[:], wqk_sb[:, kc, 4 + j, :],
                                     xts[tt][kc][:], start=(kc == 0),
                                     stop=(kc == 7)).annotate('mm_k')
                    if kc == 7:
                        nc.vector.tensor_copy(kt_store[:, j, tt, :], psk[:])

                def v_mm(sub, kc):
                    if kc == 0:
                        state[('v', sub)] = ps.tile([128, 512], dtf, tag="ps", name="psv")
                    psv = state[('v', sub)]
                    nc.tensor.matmul(psv[:],
                                     xts[tt][kc][:, sub * 128:(sub + 1) * 128],
                                     wv_sb[:, kc, :], start=(kc == 0),
                                     stop=(kc == 7)).annotate('mm_v')
                    if kc == 7:
                        vt = tt * 4 + sub
                        nc.vector.tensor_copy(
                            v_all[:, vt, :, 0:64],
                            psv[:].rearrange("p (h d) -> p h d", h=8))

                for j in range(4):
                    for kc in range(8):
                        cl.append(lambda j=j, kc=kc: q_mm(j, kc))
                for j in range(4):
                    for kc in range(8):
                        cl.append(lambda j=j, kc=kc: k_mm(j, kc))
                for sub in range(4):
                    for kc in range(8):
                        cl.append(lambda s=sub, kc=kc: v_mm(s, kc))
                return cl

            def y_closures(qt):
                cl = []
                state = {}

                def y_mm(mi, oc, hp):
                    if hp == 0:
                        state[(mi, oc)] = ps.tile([128, 512], dtf, tag="ps", name="psy")
                    psy = state[(mi, oc)]
                    nc.tensor.matmul(psy[:],
                                     cns[qt][hp][:, mi * 128:(mi + 1) * 128],
                                     wout_sb[:, hp, oc, :],
                                     start=(hp == 0), stop=(hp == 3)).annotate('mm_y')
                    if hp == 3:
                        y_sb = y_p.tile([128, 512], dtf, tag="y")
                        nc.vector.tensor_copy(y_sb[:], psy[:])
                        nc.sync.dma_start(
                            y_d[qt * 512 + mi * 128: qt * 512 + (mi + 1) * 128,
                                oc * 512:(oc + 1) * 512],
                            y_sb[:])

                for mi in range(4):
                    for oc in range(2):
                        for hp in range(4):
                            cl.append(lambda m=mi, o=oc, h=hp: y_mm(m, o, h))
                return cl

            def unit(qt, hp, hb, cn_t, pull):
                n_kt = 4 * qt + 4
                np_ = n_kt // 2
                pb = hb * 64
                ctx = ps_ctx.tile([65, 512], dtf, tag="ctx")
                pending = []

                def off(kt):
                    # valid q prefix offset: diagonal kt blocks only cover
                    # q >= 128*di; earlier columns are never written/read.
                    di = kt - 4 * qt
                    return 128 * di if di > 0 else 0

                def emit_ctx(j, ex):
                    for t in (0, 1):
                        kt = 2 * j + t
                        o = off(kt)
                        nc.tensor.matmul(ctx[:, o:512],
                                         v_all[:, kt, 2 * hp + hb, :],
                                         ex[:, t, o:512], start=(kt == 0),
                                         stop=(kt == n_kt - 1)).annotate('mm_ctx')

                for j in range(np_):
                    sp = ps_s.tile([128, 2, 512], dtf, tag="s")
                    for t in (0, 1):
                        kt = 2 * j + t
                        ktt, kj = kt // 4, kt % 4
                        o = off(kt)
                        nc.tensor.matmul(
                            sp[:, t, o:512],
                            kt_store[pb:pb + 64, hp, ktt,
                                     kj * 128:(kj + 1) * 128],
                            qts[qt][hp][pb:pb + 64, o:512],
                            start=True, stop=True).annotate('mm_s')
                    pull(2)
                    ex = exp_p.tile([128, 2, 512], dtb, tag="exp")
                    if off(2 * j) >= 256:
                        # narrow diagonal pair: exp only the valid slices
                        for t in (0, 1):
                            o = off(2 * j + t)
                            nc.scalar.activation(ex[:, t, o:512],
                                                 sp[:, t, o:512], EXP)
                    else:
                        nc.scalar.activation(ex[:], sp[:], EXP)
                    for t in (0, 1):
                        kt = 2 * j + t
                        di = kt - 4 * qt
                        if 0 <= di < 4:
                            o, w = 128 * di, 128 * (di + 1)
                            nc.vector.tensor_tensor(ex[:, t, o:w], ex[:, t, o:w],
                                                    masks[:, di, o:w], MULT)
                    pending.append((j, ex))
                    if len(pending) > 2:
                        emit_ctx(*pending.pop(0))
                        pull(1)
                while pending:
                    emit_ctx(*pending.pop(0))
                    pull(1)
                # normalize this head's half of cn
                row = row_p.tile([1, 512], dtf, tag="row")
                nc.vector.tensor_copy(row[:], ctx[64:65, :])
                rrow = row_p.tile([1, 512], dtf, tag="row", name="rrow")
                nc.vector.reciprocal_approx_fast(rrow[:], row[:])
                pull(1)
                rcp = rcp_p.tile([64, 512], dtf, tag="rcp")
                nc.gpsimd.partition_broadcast(rcp[:], rrow[:])
                nc.vector.tensor_tensor(cn_t[pb:pb + 64, :], ctx[0:64, :],
                                        rcp[:], MULT)

            def make_puller(items, total_slots):
                st = {'i': 0, 'slot': 0}
                n = len(items)

                def pull(k):
                    st['slot'] += k
                    if total_slots > 0:
                        target = min(n, (n * st['slot'] + total_slots - 1)
                                     // total_slots)
                    else:
                        target = n
                    while st['i'] < target:
                        items[st['i']]()
                        st['i'] += 1

                def drain():
                    while st['i'] < n:
                        items[st['i']]()
                        st['i'] += 1
                return pull, drain

            # ---- prologue: tt=0 projections ----
            # warm the exp activation table off the critical path
            warm = row_p.tile([1, 1], dtf, tag="row", name="warm")
            nc.scalar.activation(warm[:], v_all[0:1, 0, 0, 64:65], EXP)
            for c in proj_closures(0):
                c()

            # ---- main loop ----
            for qt in range(4):
                Bq = []
                if qt < 2:
                    emit_xt_dma(qt + 2)
                if qt < 3:
                    Bq += proj_closures(qt + 1)
                if qt == 3:
                    Bq += y_closures(0) + y_closures(1) + y_closures(2)
                np_ = (4 * qt + 4) // 2
                total_slots = 8 * (3 * np_ + 1)
                pull, drain = make_puller(Bq, total_slots)
                cns[qt] = []
                for hp in range(4):
                    cn_t = cn_p.tile([128, 512], dtb, tag="cn")
                    unit(qt, hp, 0, cn_t, pull)
                    unit(qt, hp, 1, cn_t, pull)
                    cns[qt].append(cn_t)
                drain()
            for c in y_closures(3):
                c()
    nc.compile()
    return nc


def _host_shards(x, Wqkv, bqkv, Wout):
    import ml_dtypes
    mask = np.zeros((4, 128, 512), np.float32)
    qq = np.arange(512)[None, :]
    kk = np.arange(128)[:, None]
    for di in range(4):
        mask[di] = (kk + di * 128 <= qq)
    mask = mask.astype(ml_dtypes.bfloat16)

    assert not np.any(bqkv), "kernel assumes zero qkv bias"

    in_maps = []
    for c in range(NCORES):
        b, hg = c // 2, c % 2
        s = hg * 512
        xt = np.ascontiguousarray(x[b].T).astype(ml_dtypes.bfloat16)
        wqk = np.ascontiguousarray(
            np.concatenate([Wqkv[:, s:s + 512] * 0.125,
                            Wqkv[:, 1024 + s:1024 + s + 512]],
                           axis=1)).astype(ml_dtypes.bfloat16)
        wv = np.ascontiguousarray(Wqkv[:, 2048 + s:2048 + s + 512]).astype(ml_dtypes.bfloat16)
        wout = np.ascontiguousarray(Wout[s:s + 512, :]).astype(ml_dtypes.bfloat16)
        in_maps.append({"xt": xt, "wqk": wqk, "wv": wv, "wout": wout,
                        "mask": mask})
    return in_maps


_CACHED = {}


def kernel(x, Wqkv, bqkv, Wout, bout):
    from concourse.bass_utils import run_bass_kernel_spmd

    x = np.asarray(x, dtype=np.float32)
    Wqkv = np.asarray(Wqkv, dtype=np.float32)
    bqkv = np.asarray(bqkv, dtype=np.float32)
    Wout = np.asarray(Wout, dtype=np.float32)
    bout = np.asarray(bout, dtype=np.float32)
    assert x.shape == (B, T, C), x.shape

    if 'nc' not in _CACHED:
        _CACHED['nc'] = _build_program()
    nc = _CACHED['nc']

    in_maps = _host_shards(x, Wqkv, bqkv, Wout)
    res = run_bass_kernel_spmd(nc, in_maps, core_ids=list(range(NCORES)))

    y = np.empty((B, T, C), np.float32)
    for b in range(B):
        y[b] = res.results[2 * b]["y"] + res.results[2 * b + 1]["y"] + bout
    return y
